# revision 13
# baseline (speedup 1.0000x reference)
"""Trainium2 Bass kernel for DetCenterDense: shared 3x3 conv + ReLU + four 1x1
head convs (cls/box/dir/scr, sigmoid on scr), output concatenated on channels.

Full inputs in / full output out. Sharding: 8 cores = batch(4) x H-halves(2).
Each core computes a [20, 256, 512] output shard from a [128, 258, 512]
haloed input shard.

Per-core compute: the 3x3 conv is 9 shifted 1x1 convs accumulated in PSUM.
Output rows are processed in pairs packed into one PSUM tile [128, 512]
(partitions 0:64 = row y, 64:128 = row y+1), which lets most matmuls run with
M=128 (packed weights [W_a | W_b]) instead of M=64, and gives the head conv a
full K=128 contraction via block-diagonal head weights.
"""

import numpy as np

HS = 256          # output rows per core shard
HALO = HS + 2     # input rows per core shard (1-row halo each side)
W = 512
CH = 4            # input rows per DMA chunk
NCHUNK = (HALO + CH - 1) // CH

_NC_CACHE = {}


def _build_nc():
    from contextlib import ExitStack

    import concourse.mybir as mybir
    import concourse.tile as tile
    from concourse import bacc

    f32 = mybir.dt.float32
    bf16 = mybir.dt.bfloat16
    Sigmoid = mybir.ActivationFunctionType.Sigmoid

    nc = bacc.Bacc("TRN2", target_bir_lowering=False, debug=False, num_devices=8)
    x_d = nc.dram_tensor("x", [128, HALO * W], f32, kind="ExternalInput").ap()
    wf_d = nc.dram_tensor("wfull", [128, 6 * 128], bf16, kind="ExternalInput").ap()
    wh_d = nc.dram_tensor("whalf", [128, 6 * 64], bf16, kind="ExternalInput").ap()
    whd_d = nc.dram_tensor("wheads", [128, 40], bf16, kind="ExternalInput").ap()
    b_d = nc.dram_tensor("b40", [40, 1], f32, kind="ExternalInput").ap()
    out_d = nc.dram_tensor("out", [20, HS * W], f32, kind="ExternalOutput").ap()

    with ExitStack() as ctx:
        tc = ctx.enter_context(tile.TileContext(nc))
        wpool = ctx.enter_context(tc.tile_pool(name="w", bufs=1))
        inpool = ctx.enter_context(tc.tile_pool(name="xin", bufs=5))
        bfpool = ctx.enter_context(tc.tile_pool(name="xbf", bufs=5))
        xrpool = ctx.enter_context(tc.tile_pool(name="xr", bufs=3))
        opool = ctx.enter_context(tc.tile_pool(name="ot", bufs=4))
        ppool = ctx.enter_context(tc.tile_pool(name="pp", bufs=3, space="PSUM"))
        hpool = ctx.enter_context(tc.tile_pool(name="hp", bufs=2, space="PSUM"))

        wf = wpool.tile([128, 6 * 128], bf16)
        nc.sync.dma_start(wf[:], wf_d[:])
        wh = wpool.tile([128, 6 * 64], bf16)
        nc.sync.dma_start(wh[:], wh_d[:])
        whd = wpool.tile([128, 40], bf16)
        nc.sync.dma_start(whd[:], whd_d[:])
        bt = wpool.tile([40, 1], f32)
        nc.sync.dma_start(bt[:], b_d[:])

        chunks = [None] * NCHUNK

        def load_chunk(c):
            r0 = c * CH
            rows = min(CH, HALO - r0)
            n = rows * W
            t = inpool.tile([128, CH * W], f32, tag="chunk")
            nc.sync.dma_start(t[:, 0:n], x_d[:, r0 * W : r0 * W + n])
            xb = bfpool.tile([128, CH * W], bf16, tag="xb")
            nc.gpsimd.tensor_copy(xb[:, 0:n], t[:, 0:n])
            chunks[c] = xb

        # per-tap column windows: out[:, so0:so1] += W_kx^T @ in[:, si0:si1]
        CUTS = {0: (0, 511, 1, 512), 1: (0, 512, 0, 512), 2: (1, 512, 0, 511)}

        def row_slice(j, si0, si1):
            t = chunks[j // CH]
            o = (j % CH) * W
            return t[:, o + si0 : o + si1]

        loaded = 0
        for p in range(HS // 2):
            cneed = (2 * p + 3) // CH
            while loaded <= min(cneed + 1, NCHUNK - 1):
                load_chunk(loaded)
                loaded += 1

            P = ppool.tile([128, W], f32, tag="pp")
            a, b, c, d = 2 * p, 2 * p + 1, 2 * p + 2, 2 * p + 3
            first = True
            # full-width matmuls: rows b ([W1|W0]) and c ([W2|W1])
            for kx in (1, 0, 2):
                si0, si1, so0, so1 = CUTS[kx]
                for t_idx, j in ((0, b), (1, c)):
                    blk = 2 * kx + t_idx
                    nc.tensor.matmul(
                        P[:, so0:so1],
                        wf[:, blk * 128 : (blk + 1) * 128],
                        row_slice(j, si0, si1),
                        start=first,
                        stop=False,
                    )
                    first = False
            # half matmuls: row a (W0 -> psum[0:64]) and row d (W2 -> psum[64:128])
            for kx in (1, 0, 2):
                si0, si1, so0, so1 = CUTS[kx]
                last = kx == 2
                nc.tensor.matmul(
                    P[0:64, so0:so1],
                    wh[:, (2 * kx) * 64 : (2 * kx + 1) * 64],
                    row_slice(a, si0, si1),
                    start=False,
                    stop=last,
                )
                nc.tensor.matmul(
                    P[64:128, so0:so1],
                    wh[:, (2 * kx + 1) * 64 : (2 * kx + 2) * 64],
                    row_slice(d, si0, si1),
                    start=False,
                    stop=last,
                )

            xr = xrpool.tile([128, W], bf16, tag="xr")
            nc.vector.tensor_scalar_max(xr[:], P[:], 0.0)

            hP = hpool.tile([40, W], f32, tag="hp")
            nc.tensor.matmul(hP[:], whd[:], xr[:], start=True, stop=True)

            ot = opool.tile([40, W], f32, tag="ot")
            nc.vector.tensor_scalar_add(ot[:], hP[:], bt[:])
            nc.scalar.activation(ot[32:40, :], hP[32:40, :], Sigmoid, bias=bt[32:40, :])

            y = 2 * p
            nc.sync.dma_start(out_d[0:16, y * W : (y + 1) * W], ot[0:16, :])
            nc.sync.dma_start(out_d[16:20, y * W : (y + 1) * W], ot[32:36, :])
            nc.sync.dma_start(out_d[0:16, (y + 1) * W : (y + 2) * W], ot[16:32, :])
            nc.sync.dma_start(out_d[16:20, (y + 1) * W : (y + 2) * W], ot[36:40, :])

    nc.compile()
    return nc


def _get_nc():
    if "nc" not in _NC_CACHE:
        _NC_CACHE["nc"] = _build_nc()
    return _NC_CACHE["nc"]


def _pack_weights(w_shared, w_cls, b_cls, w_box, b_box, w_dir, b_dir, w_scr, b_scr):
    Wt = np.ascontiguousarray(w_shared, np.float32).transpose(1, 0, 2, 3)  # [128,64,3,3]
    wfull = np.zeros((128, 6, 128), np.float32)
    whalf = np.zeros((128, 6, 64), np.float32)
    for kx in range(3):
        wfull[:, 2 * kx + 0, 0:64] = Wt[:, :, 1, kx]
        wfull[:, 2 * kx + 0, 64:128] = Wt[:, :, 0, kx]
        wfull[:, 2 * kx + 1, 0:64] = Wt[:, :, 2, kx]
        wfull[:, 2 * kx + 1, 64:128] = Wt[:, :, 1, kx]
        whalf[:, 2 * kx + 0] = Wt[:, :, 0, kx]
        whalf[:, 2 * kx + 1] = Wt[:, :, 2, kx]
    import ml_dtypes

    wfull = np.ascontiguousarray(wfull.reshape(128, 768)).astype(ml_dtypes.bfloat16)
    whalf = np.ascontiguousarray(whalf.reshape(128, 384)).astype(ml_dtypes.bfloat16)

    Wh = np.concatenate([w_cls, w_box, w_dir, w_scr], 0)[:, :, 0, 0].astype(np.float32)  # [20,64]
    bh = np.concatenate([b_cls, b_box, b_dir, b_scr], 0).astype(np.float32)  # [20]
    # head-output partition layout (sigmoid rows 32-aligned for ACT):
    #   0:16  row-y   cls/box/dir        (k 0:64)
    #   16:32 row-y+1 cls/box/dir        (k 64:128)
    #   32:36 row-y   scr                (k 0:64)
    #   36:40 row-y+1 scr                (k 64:128)
    wheads = np.zeros((128, 40), np.float32)
    wheads[0:64, 0:16] = Wh[0:16].T
    wheads[64:128, 16:32] = Wh[0:16].T
    wheads[0:64, 32:36] = Wh[16:20].T
    wheads[64:128, 36:40] = Wh[16:20].T
    wheads = wheads.astype(ml_dtypes.bfloat16)
    b40 = np.ascontiguousarray(
        np.concatenate([bh[0:16], bh[0:16], bh[16:20], bh[16:20]])[:, None]
    )  # [40,1]
    return wfull, whalf, wheads, b40


def kernel(**inputs):
    from concourse.bass_utils import run_bass_kernel_spmd

    feature = np.ascontiguousarray(inputs["feature"], np.float32)  # [4,128,512,512]
    B, Cin, H, Wd = feature.shape
    assert (B, Cin, H, Wd) == (4, 128, 512, 512)

    wfull, whalf, wheads, b40 = _pack_weights(
        np.asarray(inputs["w_shared"]),
        np.asarray(inputs["w_cls"]), np.asarray(inputs["b_cls"]),
        np.asarray(inputs["w_box"]), np.asarray(inputs["b_box"]),
        np.asarray(inputs["w_dir"]), np.asarray(inputs["b_dir"]),
        np.asarray(inputs["w_scr"]), np.asarray(inputs["b_scr"]),
    )

    in_maps = []
    for core in range(8):
        bi, half = core // 2, core % 2
        r0 = half * HS
        xs = np.zeros((128, HALO, W), np.float32)
        lo, hi = r0 - 1, r0 + HS + 1
        slo, shi = max(lo, 0), min(hi, H)
        xs[:, slo - lo : HALO - (hi - shi), :] = feature[bi, :, slo:shi, :]
        in_maps.append(
            {
                "x": xs.reshape(128, HALO * W),
                "wfull": wfull,
                "whalf": whalf,
                "wheads": wheads,
                "b40": b40,
            }
        )

    nc = _get_nc()
    res = run_bass_kernel_spmd(nc, in_maps, core_ids=list(range(8)))

    out = np.empty((4, 20, 512, 512), np.float32)
    for core in range(8):
        bi, half = core // 2, core % 2
        out[bi, :, half * HS : (half + 1) * HS, :] = res.results[core]["out"].reshape(
            20, HS, W
        )
    return out


def run_traced(**inputs):
    """Like kernel(), but returns (out, BassKernelResults) with a profile trace."""
    from concourse.bass_utils import run_bass_kernel_spmd

    feature = np.ascontiguousarray(inputs["feature"], np.float32)
    wfull, whalf, wheads, b40 = _pack_weights(
        np.asarray(inputs["w_shared"]),
        np.asarray(inputs["w_cls"]), np.asarray(inputs["b_cls"]),
        np.asarray(inputs["w_box"]), np.asarray(inputs["b_box"]),
        np.asarray(inputs["w_dir"]), np.asarray(inputs["b_dir"]),
        np.asarray(inputs["w_scr"]), np.asarray(inputs["b_scr"]),
    )
    in_maps = []
    for core in range(8):
        bi, half = core // 2, core % 2
        r0 = half * HS
        xs = np.zeros((128, HALO, W), np.float32)
        lo, hi = r0 - 1, r0 + HS + 1
        slo, shi = max(lo, 0), min(hi, 512)
        xs[:, slo - lo : HALO - (hi - shi), :] = feature[bi, :, slo:shi, :]
        in_maps.append(
            {"x": xs.reshape(128, HALO * W), "wfull": wfull, "whalf": whalf,
             "wheads": wheads, "b40": b40}
        )
    nc = _get_nc()
    res = run_bass_kernel_spmd(nc, in_maps, core_ids=list(range(8)), trace=True)
    out = np.empty((4, 20, 512, 512), np.float32)
    for core in range(8):
        bi, half = core // 2, core % 2
        out[bi, :, half * HS : (half + 1) * HS, :] = res.results[core]["out"].reshape(
            20, HS, W
        )
    return out, res


# revision 16
# speedup vs baseline: 1.0541x; 1.0541x over previous
"""Trainium2 Bass kernel for DetCenterDense: shared 3x3 conv + ReLU + four 1x1
head convs (cls/box/dir/scr, sigmoid on scr), output concatenated on channels.

Full inputs in / full output out. Sharding: 8 cores = batch(4) x H-halves(2).
Each core computes a [20, 256, 512] output shard from a [128, 258, 512]
haloed input shard.

Per-core compute: the 3x3 conv is 9 shifted 1x1 convs accumulated in PSUM.
Output rows are processed in pairs packed into one PSUM tile [128, 512]
(partitions 0:64 = row y, 64:128 = row y+1), which lets most matmuls run with
M=128 (packed weights [W_a | W_b]) instead of M=64, and gives the head conv a
full K=128 contraction via block-diagonal head weights.
"""

import numpy as np

HS = 256          # output rows per core shard
HALO = HS + 2     # input rows per core shard (1-row halo each side)
W = 512
CH = 4            # input rows per DMA chunk
NCHUNK = (HALO + CH - 1) // CH

_NC_CACHE = {}


def _build_nc():
    from contextlib import ExitStack

    import concourse.mybir as mybir
    import concourse.tile as tile
    from concourse import bacc

    f32 = mybir.dt.float32
    bf16 = mybir.dt.bfloat16
    Sigmoid = mybir.ActivationFunctionType.Sigmoid

    nc = bacc.Bacc("TRN2", target_bir_lowering=False, debug=False, num_devices=8)
    x_d = nc.dram_tensor("x", [128, HALO * W], f32, kind="ExternalInput").ap()
    wf_d = nc.dram_tensor("wfull", [128, 6 * 128], bf16, kind="ExternalInput").ap()
    wh_d = nc.dram_tensor("whalf", [128, 6 * 64], bf16, kind="ExternalInput").ap()
    whd_d = nc.dram_tensor("wheads", [128, 40], bf16, kind="ExternalInput").ap()
    b_d = nc.dram_tensor("b40", [40, 1], f32, kind="ExternalInput").ap()
    out_d = nc.dram_tensor("out", [20, HS * W], f32, kind="ExternalOutput").ap()

    with ExitStack() as ctx:
        tc = ctx.enter_context(tile.TileContext(nc))
        wpool = ctx.enter_context(tc.tile_pool(name="w", bufs=1))
        inpool = ctx.enter_context(tc.tile_pool(name="xin", bufs=6))
        bfpool = ctx.enter_context(tc.tile_pool(name="xbf", bufs=6))
        xrpool = ctx.enter_context(tc.tile_pool(name="xr", bufs=3))
        opool = ctx.enter_context(tc.tile_pool(name="ot", bufs=4))
        ppool = ctx.enter_context(tc.tile_pool(name="pp", bufs=3, space="PSUM"))
        hpool = ctx.enter_context(tc.tile_pool(name="hp", bufs=2, space="PSUM"))

        wf = wpool.tile([128, 6 * 128], bf16)
        nc.sync.dma_start(wf[:], wf_d[:])
        wh = wpool.tile([128, 6 * 64], bf16)
        nc.sync.dma_start(wh[:], wh_d[:])
        whd = wpool.tile([128, 40], bf16)
        nc.sync.dma_start(whd[:], whd_d[:])
        bt = wpool.tile([40, 1], f32)
        nc.sync.dma_start(bt[:], b_d[:])

        chunks = [None] * NCHUNK

        def load_chunk(c):
            r0 = c * CH
            rows = min(CH, HALO - r0)
            n = rows * W
            t = inpool.tile([128, CH * W], f32, tag="chunk")
            nc.sync.dma_start(t[:, 0:n], x_d[:, r0 * W : r0 * W + n])
            xb = bfpool.tile([128, CH * W], bf16, tag="xb")
            nc.scalar.copy(xb[:, 0:n], t[:, 0:n])
            chunks[c] = xb

        # per-tap column windows: out[:, so0:so1] += W_kx^T @ in[:, si0:si1]
        CUTS = {0: (0, 511, 1, 512), 1: (0, 512, 0, 512), 2: (1, 512, 0, 511)}

        def row_slice(j, si0, si1):
            t = chunks[j // CH]
            o = (j % CH) * W
            return t[:, o + si0 : o + si1]

        loaded = 0
        for p in range(HS // 2):
            cneed = (2 * p + 3) // CH
            while loaded <= min(cneed + 2, NCHUNK - 1):
                load_chunk(loaded)
                loaded += 1

            P = ppool.tile([128, W], f32, tag="pp")
            a, b, c, d = 2 * p, 2 * p + 1, 2 * p + 2, 2 * p + 3
            first = True
            # full-width matmuls: rows b ([W1|W0]) and c ([W2|W1])
            for kx in (1, 0, 2):
                si0, si1, so0, so1 = CUTS[kx]
                for t_idx, j in ((0, b), (1, c)):
                    blk = 2 * kx + t_idx
                    nc.tensor.matmul(
                        P[:, so0:so1],
                        wf[:, blk * 128 : (blk + 1) * 128],
                        row_slice(j, si0, si1),
                        start=first,
                        stop=False,
                    )
                    first = False
            # half matmuls: row a (W0 -> psum[0:64]) and row d (W2 -> psum[64:128])
            for kx in (1, 0, 2):
                si0, si1, so0, so1 = CUTS[kx]
                last = kx == 2
                nc.tensor.matmul(
                    P[0:64, so0:so1],
                    wh[:, (2 * kx) * 64 : (2 * kx + 1) * 64],
                    row_slice(a, si0, si1),
                    start=False,
                    stop=last,
                )
                nc.tensor.matmul(
                    P[64:128, so0:so1],
                    wh[:, (2 * kx + 1) * 64 : (2 * kx + 2) * 64],
                    row_slice(d, si0, si1),
                    start=False,
                    stop=last,
                )

            xr = xrpool.tile([128, W], bf16, tag="xr")
            nc.vector.tensor_scalar_max(xr[:], P[:], 0.0)

            hP = hpool.tile([40, W], f32, tag="hp")
            nc.tensor.matmul(hP[:], whd[:], xr[:], start=True, stop=True)

            ot = opool.tile([40, W], f32, tag="ot")
            nc.vector.tensor_scalar_add(ot[:], hP[:], bt[:])
            nc.scalar.activation(ot[32:40, :], hP[32:40, :], Sigmoid, bias=bt[32:40, :])

            y = 2 * p
            nc.sync.dma_start(out_d[0:16, y * W : (y + 1) * W], ot[0:16, :])
            nc.sync.dma_start(out_d[16:20, y * W : (y + 1) * W], ot[32:36, :])
            nc.sync.dma_start(out_d[0:16, (y + 1) * W : (y + 2) * W], ot[16:32, :])
            nc.sync.dma_start(out_d[16:20, (y + 1) * W : (y + 2) * W], ot[36:40, :])

    nc.compile()
    return nc


def _get_nc():
    if "nc" not in _NC_CACHE:
        _NC_CACHE["nc"] = _build_nc()
    return _NC_CACHE["nc"]


def _pack_weights(w_shared, w_cls, b_cls, w_box, b_box, w_dir, b_dir, w_scr, b_scr):
    Wt = np.ascontiguousarray(w_shared, np.float32).transpose(1, 0, 2, 3)  # [128,64,3,3]
    wfull = np.zeros((128, 6, 128), np.float32)
    whalf = np.zeros((128, 6, 64), np.float32)
    for kx in range(3):
        wfull[:, 2 * kx + 0, 0:64] = Wt[:, :, 1, kx]
        wfull[:, 2 * kx + 0, 64:128] = Wt[:, :, 0, kx]
        wfull[:, 2 * kx + 1, 0:64] = Wt[:, :, 2, kx]
        wfull[:, 2 * kx + 1, 64:128] = Wt[:, :, 1, kx]
        whalf[:, 2 * kx + 0] = Wt[:, :, 0, kx]
        whalf[:, 2 * kx + 1] = Wt[:, :, 2, kx]
    import ml_dtypes

    wfull = np.ascontiguousarray(wfull.reshape(128, 768)).astype(ml_dtypes.bfloat16)
    whalf = np.ascontiguousarray(whalf.reshape(128, 384)).astype(ml_dtypes.bfloat16)

    Wh = np.concatenate([w_cls, w_box, w_dir, w_scr], 0)[:, :, 0, 0].astype(np.float32)  # [20,64]
    bh = np.concatenate([b_cls, b_box, b_dir, b_scr], 0).astype(np.float32)  # [20]
    # head-output partition layout (sigmoid rows 32-aligned for ACT):
    #   0:16  row-y   cls/box/dir        (k 0:64)
    #   16:32 row-y+1 cls/box/dir        (k 64:128)
    #   32:36 row-y   scr                (k 0:64)
    #   36:40 row-y+1 scr                (k 64:128)
    wheads = np.zeros((128, 40), np.float32)
    wheads[0:64, 0:16] = Wh[0:16].T
    wheads[64:128, 16:32] = Wh[0:16].T
    wheads[0:64, 32:36] = Wh[16:20].T
    wheads[64:128, 36:40] = Wh[16:20].T
    wheads = wheads.astype(ml_dtypes.bfloat16)
    b40 = np.ascontiguousarray(
        np.concatenate([bh[0:16], bh[0:16], bh[16:20], bh[16:20]])[:, None]
    )  # [40,1]
    return wfull, whalf, wheads, b40


def kernel(**inputs):
    from concourse.bass_utils import run_bass_kernel_spmd

    feature = np.ascontiguousarray(inputs["feature"], np.float32)  # [4,128,512,512]
    B, Cin, H, Wd = feature.shape
    assert (B, Cin, H, Wd) == (4, 128, 512, 512)

    wfull, whalf, wheads, b40 = _pack_weights(
        np.asarray(inputs["w_shared"]),
        np.asarray(inputs["w_cls"]), np.asarray(inputs["b_cls"]),
        np.asarray(inputs["w_box"]), np.asarray(inputs["b_box"]),
        np.asarray(inputs["w_dir"]), np.asarray(inputs["b_dir"]),
        np.asarray(inputs["w_scr"]), np.asarray(inputs["b_scr"]),
    )

    in_maps = []
    for core in range(8):
        bi, half = core // 2, core % 2
        r0 = half * HS
        xs = np.zeros((128, HALO, W), np.float32)
        lo, hi = r0 - 1, r0 + HS + 1
        slo, shi = max(lo, 0), min(hi, H)
        xs[:, slo - lo : HALO - (hi - shi), :] = feature[bi, :, slo:shi, :]
        in_maps.append(
            {
                "x": xs.reshape(128, HALO * W),
                "wfull": wfull,
                "whalf": whalf,
                "wheads": wheads,
                "b40": b40,
            }
        )

    nc = _get_nc()
    res = run_bass_kernel_spmd(nc, in_maps, core_ids=list(range(8)))

    out = np.empty((4, 20, 512, 512), np.float32)
    for core in range(8):
        bi, half = core // 2, core % 2
        out[bi, :, half * HS : (half + 1) * HS, :] = res.results[core]["out"].reshape(
            20, HS, W
        )
    return out


def run_traced(**inputs):
    """Like kernel(), but returns (out, BassKernelResults) with a profile trace."""
    from concourse.bass_utils import run_bass_kernel_spmd

    feature = np.ascontiguousarray(inputs["feature"], np.float32)
    wfull, whalf, wheads, b40 = _pack_weights(
        np.asarray(inputs["w_shared"]),
        np.asarray(inputs["w_cls"]), np.asarray(inputs["b_cls"]),
        np.asarray(inputs["w_box"]), np.asarray(inputs["b_box"]),
        np.asarray(inputs["w_dir"]), np.asarray(inputs["b_dir"]),
        np.asarray(inputs["w_scr"]), np.asarray(inputs["b_scr"]),
    )
    in_maps = []
    for core in range(8):
        bi, half = core // 2, core % 2
        r0 = half * HS
        xs = np.zeros((128, HALO, W), np.float32)
        lo, hi = r0 - 1, r0 + HS + 1
        slo, shi = max(lo, 0), min(hi, 512)
        xs[:, slo - lo : HALO - (hi - shi), :] = feature[bi, :, slo:shi, :]
        in_maps.append(
            {"x": xs.reshape(128, HALO * W), "wfull": wfull, "whalf": whalf,
             "wheads": wheads, "b40": b40}
        )
    nc = _get_nc()
    res = run_bass_kernel_spmd(nc, in_maps, core_ids=list(range(8)), trace=True)
    out = np.empty((4, 20, 512, 512), np.float32)
    for core in range(8):
        bi, half = core // 2, core % 2
        out[bi, :, half * HS : (half + 1) * HS, :] = res.results[core]["out"].reshape(
            20, HS, W
        )
    return out, res


# revision 18
# speedup vs baseline: 1.7690x; 1.6782x over previous
"""Trainium2 Bass kernel for DetCenterDense: shared 3x3 conv + ReLU + four 1x1
head convs (cls/box/dir/scr, sigmoid on scr), output concatenated on channels.

Full inputs in / full output out. Sharding: 8 cores = batch(4) x H-halves(2).
Each core computes a [20, 256, 512] output shard from a [128, 258, 512]
haloed input shard.

Per-core compute: the 3x3 conv is 9 shifted 1x1 convs accumulated in PSUM.
Output rows are processed in pairs packed into one PSUM tile [128, 512]
(partitions 0:64 = row y, 64:128 = row y+1), which lets most matmuls run with
M=128 (packed weights [W_a | W_b]) instead of M=64, and gives the head conv a
full K=128 contraction via block-diagonal head weights.
"""

import numpy as np

HS = 256          # output rows per core shard
HALO = HS + 2     # input rows per core shard (1-row halo each side)
W = 512
CH = 8            # input rows per DMA chunk
NCHUNK = (HALO + CH - 1) // CH

_NC_CACHE = {}


def _build_nc():
    from contextlib import ExitStack

    import concourse.mybir as mybir
    import concourse.tile as tile
    from concourse import bacc

    f32 = mybir.dt.float32
    bf16 = mybir.dt.bfloat16
    Sigmoid = mybir.ActivationFunctionType.Sigmoid

    nc = bacc.Bacc("TRN2", target_bir_lowering=False, debug=False, num_devices=8)
    x_d = nc.dram_tensor("x", [128, HALO * W], f32, kind="ExternalInput").ap()
    wf_d = nc.dram_tensor("wfull", [128, 6 * 128], bf16, kind="ExternalInput").ap()
    wh_d = nc.dram_tensor("whalf", [128, 6 * 64], bf16, kind="ExternalInput").ap()
    whd_d = nc.dram_tensor("wheads", [128, 40], bf16, kind="ExternalInput").ap()
    b_d = nc.dram_tensor("b40", [40, 1], f32, kind="ExternalInput").ap()
    out_d = nc.dram_tensor("out", [20, HS * W], f32, kind="ExternalOutput").ap()

    with ExitStack() as ctx:
        tc = ctx.enter_context(tile.TileContext(nc))
        wpool = ctx.enter_context(tc.tile_pool(name="w", bufs=1))
        inpool = ctx.enter_context(tc.tile_pool(name="xin", bufs=5))
        bfpool = ctx.enter_context(tc.tile_pool(name="xbf", bufs=5))
        xrpool = ctx.enter_context(tc.tile_pool(name="xr", bufs=3))
        opool = ctx.enter_context(tc.tile_pool(name="ot", bufs=3))
        ppool = ctx.enter_context(tc.tile_pool(name="pp", bufs=3, space="PSUM"))
        hpool = ctx.enter_context(tc.tile_pool(name="hp", bufs=2, space="PSUM"))

        wf = wpool.tile([128, 6 * 128], bf16)
        nc.sync.dma_start(wf[:], wf_d[:])
        wh = wpool.tile([128, 6 * 64], bf16)
        nc.sync.dma_start(wh[:], wh_d[:])
        whd = wpool.tile([128, 40], bf16)
        nc.sync.dma_start(whd[:], whd_d[:])
        bt = wpool.tile([40, 1], f32)
        nc.sync.dma_start(bt[:], b_d[:])

        chunks = [None] * NCHUNK

        def load_chunk(c):
            r0 = c * CH
            rows = min(CH, HALO - r0)
            n = rows * W
            t = inpool.tile([128, CH * W], f32, tag="chunk")
            nc.sync.dma_start(t[:, 0:n], x_d[:, r0 * W : r0 * W + n])
            xb = bfpool.tile([128, CH * W], bf16, tag="xb")
            nc.scalar.copy(xb[:, 0:n], t[:, 0:n])
            chunks[c] = xb

        # per-tap column windows: out[:, so0:so1] += W_kx^T @ in[:, si0:si1]
        CUTS = {0: (0, 511, 1, 512), 1: (0, 512, 0, 512), 2: (1, 512, 0, 511)}

        def row_slice(j, si0, si1):
            t = chunks[j // CH]
            o = (j % CH) * W
            return t[:, o + si0 : o + si1]

        loaded = 0
        for g in range(HS // 8):  # groups of 4 row-pairs
            S = opool.tile([40, 4 * W], f32, tag="S")
            for i in range(4):
                p = 4 * g + i
                cneed = (2 * p + 3) // CH
                while loaded <= min(cneed + 2, NCHUNK - 1):
                    load_chunk(loaded)
                    loaded += 1

                P = ppool.tile([128, W], f32, tag="pp")
                a, b, c, d = 2 * p, 2 * p + 1, 2 * p + 2, 2 * p + 3
                first = True
                # full-width matmuls: rows b ([W1|W0]) and c ([W2|W1])
                for kx in (1, 0, 2):
                    si0, si1, so0, so1 = CUTS[kx]
                    for t_idx, j in ((0, b), (1, c)):
                        blk = 2 * kx + t_idx
                        nc.tensor.matmul(
                            P[:, so0:so1],
                            wf[:, blk * 128 : (blk + 1) * 128],
                            row_slice(j, si0, si1),
                            start=first,
                            stop=False,
                        )
                        first = False
                # half matmuls: row a (W0 -> psum[0:64]), row d (W2 -> psum[64:128])
                for kx in (1, 0, 2):
                    si0, si1, so0, so1 = CUTS[kx]
                    last = kx == 2
                    nc.tensor.matmul(
                        P[0:64, so0:so1],
                        wh[:, (2 * kx) * 64 : (2 * kx + 1) * 64],
                        row_slice(a, si0, si1),
                        start=False,
                        stop=last,
                    )
                    nc.tensor.matmul(
                        P[64:128, so0:so1],
                        wh[:, (2 * kx + 1) * 64 : (2 * kx + 2) * 64],
                        row_slice(d, si0, si1),
                        start=False,
                        stop=last,
                    )

                xr = xrpool.tile([128, W], bf16, tag="xr")
                nc.vector.tensor_scalar_max(xr[:], P[:], 0.0)

                hP = hpool.tile([40, W], f32, tag="hp")
                nc.tensor.matmul(hP[:], whd[:], xr[:], start=True, stop=True)

                nc.vector.tensor_scalar_add(S[:, i * W : (i + 1) * W], hP[:], bt[:])
                nc.scalar.activation(
                    S[32:40, i * W : (i + 1) * W], hP[32:40, :], Sigmoid, bias=bt[32:40, :]
                )

            # 4 batched output DMAs per group (even/odd rows x nonscr/scr),
            # issued on the idle gpsimd engine (SWDGE) to keep the sync queue free for
            # input prefetch.
            y0 = 8 * g
            Sv = S[:].rearrange("q (i w) -> q i w", w=W)  # [40, 4, 512]
            ov = out_d[:, y0 * W : (y0 + 8) * W].rearrange(
                "q (i e w) -> q i e w", i=4, w=W
            )  # [20, 4, 2, 512]
            nc.gpsimd.dma_start(ov[0:16, :, 0, :], Sv[0:16])
            nc.gpsimd.dma_start(ov[0:16, :, 1, :], Sv[16:32])
            nc.gpsimd.dma_start(ov[16:20, :, 0, :], Sv[32:36])
            nc.gpsimd.dma_start(ov[16:20, :, 1, :], Sv[36:40])

    nc.compile()
    return nc


def _get_nc():
    if "nc" not in _NC_CACHE:
        _NC_CACHE["nc"] = _build_nc()
    return _NC_CACHE["nc"]


def _pack_weights(w_shared, w_cls, b_cls, w_box, b_box, w_dir, b_dir, w_scr, b_scr):
    Wt = np.ascontiguousarray(w_shared, np.float32).transpose(1, 0, 2, 3)  # [128,64,3,3]
    wfull = np.zeros((128, 6, 128), np.float32)
    whalf = np.zeros((128, 6, 64), np.float32)
    for kx in range(3):
        wfull[:, 2 * kx + 0, 0:64] = Wt[:, :, 1, kx]
        wfull[:, 2 * kx + 0, 64:128] = Wt[:, :, 0, kx]
        wfull[:, 2 * kx + 1, 0:64] = Wt[:, :, 2, kx]
        wfull[:, 2 * kx + 1, 64:128] = Wt[:, :, 1, kx]
        whalf[:, 2 * kx + 0] = Wt[:, :, 0, kx]
        whalf[:, 2 * kx + 1] = Wt[:, :, 2, kx]
    import ml_dtypes

    wfull = np.ascontiguousarray(wfull.reshape(128, 768)).astype(ml_dtypes.bfloat16)
    whalf = np.ascontiguousarray(whalf.reshape(128, 384)).astype(ml_dtypes.bfloat16)

    Wh = np.concatenate([w_cls, w_box, w_dir, w_scr], 0)[:, :, 0, 0].astype(np.float32)  # [20,64]
    bh = np.concatenate([b_cls, b_box, b_dir, b_scr], 0).astype(np.float32)  # [20]
    # head-output partition layout (sigmoid rows 32-aligned for ACT):
    #   0:16  row-y   cls/box/dir        (k 0:64)
    #   16:32 row-y+1 cls/box/dir        (k 64:128)
    #   32:36 row-y   scr                (k 0:64)
    #   36:40 row-y+1 scr                (k 64:128)
    wheads = np.zeros((128, 40), np.float32)
    wheads[0:64, 0:16] = Wh[0:16].T
    wheads[64:128, 16:32] = Wh[0:16].T
    wheads[0:64, 32:36] = Wh[16:20].T
    wheads[64:128, 36:40] = Wh[16:20].T
    wheads = wheads.astype(ml_dtypes.bfloat16)
    b40 = np.ascontiguousarray(
        np.concatenate([bh[0:16], bh[0:16], bh[16:20], bh[16:20]])[:, None]
    )  # [40,1]
    return wfull, whalf, wheads, b40


def kernel(**inputs):
    from concourse.bass_utils import run_bass_kernel_spmd

    feature = np.ascontiguousarray(inputs["feature"], np.float32)  # [4,128,512,512]
    B, Cin, H, Wd = feature.shape
    assert (B, Cin, H, Wd) == (4, 128, 512, 512)

    wfull, whalf, wheads, b40 = _pack_weights(
        np.asarray(inputs["w_shared"]),
        np.asarray(inputs["w_cls"]), np.asarray(inputs["b_cls"]),
        np.asarray(inputs["w_box"]), np.asarray(inputs["b_box"]),
        np.asarray(inputs["w_dir"]), np.asarray(inputs["b_dir"]),
        np.asarray(inputs["w_scr"]), np.asarray(inputs["b_scr"]),
    )

    in_maps = []
    for core in range(8):
        bi, half = core // 2, core % 2
        r0 = half * HS
        xs = np.zeros((128, HALO, W), np.float32)
        lo, hi = r0 - 1, r0 + HS + 1
        slo, shi = max(lo, 0), min(hi, H)
        xs[:, slo - lo : HALO - (hi - shi), :] = feature[bi, :, slo:shi, :]
        in_maps.append(
            {
                "x": xs.reshape(128, HALO * W),
                "wfull": wfull,
                "whalf": whalf,
                "wheads": wheads,
                "b40": b40,
            }
        )

    nc = _get_nc()
    res = run_bass_kernel_spmd(nc, in_maps, core_ids=list(range(8)))

    out = np.empty((4, 20, 512, 512), np.float32)
    for core in range(8):
        bi, half = core // 2, core % 2
        out[bi, :, half * HS : (half + 1) * HS, :] = res.results[core]["out"].reshape(
            20, HS, W
        )
    return out


def run_traced(**inputs):
    """Like kernel(), but returns (out, BassKernelResults) with a profile trace."""
    from concourse.bass_utils import run_bass_kernel_spmd

    feature = np.ascontiguousarray(inputs["feature"], np.float32)
    wfull, whalf, wheads, b40 = _pack_weights(
        np.asarray(inputs["w_shared"]),
        np.asarray(inputs["w_cls"]), np.asarray(inputs["b_cls"]),
        np.asarray(inputs["w_box"]), np.asarray(inputs["b_box"]),
        np.asarray(inputs["w_dir"]), np.asarray(inputs["b_dir"]),
        np.asarray(inputs["w_scr"]), np.asarray(inputs["b_scr"]),
    )
    in_maps = []
    for core in range(8):
        bi, half = core // 2, core % 2
        r0 = half * HS
        xs = np.zeros((128, HALO, W), np.float32)
        lo, hi = r0 - 1, r0 + HS + 1
        slo, shi = max(lo, 0), min(hi, 512)
        xs[:, slo - lo : HALO - (hi - shi), :] = feature[bi, :, slo:shi, :]
        in_maps.append(
            {"x": xs.reshape(128, HALO * W), "wfull": wfull, "whalf": whalf,
             "wheads": wheads, "b40": b40}
        )
    nc = _get_nc()
    res = run_bass_kernel_spmd(nc, in_maps, core_ids=list(range(8)), trace=True)
    out = np.empty((4, 20, 512, 512), np.float32)
    for core in range(8):
        bi, half = core // 2, core % 2
        out[bi, :, half * HS : (half + 1) * HS, :] = res.results[core]["out"].reshape(
            20, HS, W
        )
    return out, res


# revision 19
# speedup vs baseline: 1.8236x; 1.0309x over previous
"""Trainium2 Bass kernel for DetCenterDense: shared 3x3 conv + ReLU + four 1x1
head convs (cls/box/dir/scr, sigmoid on scr), output concatenated on channels.

Full inputs in / full output out. Sharding: 8 cores = batch(4) x H-halves(2).
Each core computes a [20, 256, 512] output shard from a [128, 258, 512]
haloed input shard.

Per-core compute: the 3x3 conv is 9 shifted 1x1 convs accumulated in PSUM.
Output rows are processed in pairs packed into one PSUM tile [128, 512]
(partitions 0:64 = row y, 64:128 = row y+1), which lets most matmuls run with
M=128 (packed weights [W_a | W_b]) instead of M=64, and gives the head conv a
full K=128 contraction via block-diagonal head weights.
"""

import numpy as np

HS = 256          # output rows per core shard
HALO = HS + 2     # input rows per core shard (1-row halo each side)
W = 512
CH = 4            # input rows per DMA chunk
NCHUNK = (HALO + CH - 1) // CH

_NC_CACHE = {}


def _build_nc():
    from contextlib import ExitStack

    import concourse.mybir as mybir
    import concourse.tile as tile
    from concourse import bacc

    f32 = mybir.dt.float32
    bf16 = mybir.dt.bfloat16
    Sigmoid = mybir.ActivationFunctionType.Sigmoid

    nc = bacc.Bacc("TRN2", target_bir_lowering=False, debug=False, num_devices=8)
    x_d = nc.dram_tensor("x", [128, HALO * W], f32, kind="ExternalInput").ap()
    wf_d = nc.dram_tensor("wfull", [128, 6 * 128], bf16, kind="ExternalInput").ap()
    wh_d = nc.dram_tensor("whalf", [128, 6 * 64], bf16, kind="ExternalInput").ap()
    whd_d = nc.dram_tensor("wheads", [128, 40], bf16, kind="ExternalInput").ap()
    b_d = nc.dram_tensor("b40", [40, 1], f32, kind="ExternalInput").ap()
    out_d = nc.dram_tensor("out", [20, HS * W], f32, kind="ExternalOutput").ap()

    with ExitStack() as ctx:
        tc = ctx.enter_context(tile.TileContext(nc))
        wpool = ctx.enter_context(tc.tile_pool(name="w", bufs=1))
        inpool = ctx.enter_context(tc.tile_pool(name="xin", bufs=8))
        bfpool = ctx.enter_context(tc.tile_pool(name="xbf", bufs=8))
        xrpool = ctx.enter_context(tc.tile_pool(name="xr", bufs=3))
        opool = ctx.enter_context(tc.tile_pool(name="ot", bufs=3))
        ppool = ctx.enter_context(tc.tile_pool(name="pp", bufs=3, space="PSUM"))
        hpool = ctx.enter_context(tc.tile_pool(name="hp", bufs=2, space="PSUM"))

        wf = wpool.tile([128, 6 * 128], bf16)
        nc.sync.dma_start(wf[:], wf_d[:])
        wh = wpool.tile([128, 6 * 64], bf16)
        nc.sync.dma_start(wh[:], wh_d[:])
        whd = wpool.tile([128, 40], bf16)
        nc.sync.dma_start(whd[:], whd_d[:])
        bt = wpool.tile([40, 1], f32)
        nc.sync.dma_start(bt[:], b_d[:])

        chunks = [None] * NCHUNK

        def load_chunk(c):
            r0 = c * CH
            rows = min(CH, HALO - r0)
            n = rows * W
            t = inpool.tile([128, CH * W], f32, tag="chunk")
            nc.sync.dma_start(t[:, 0:n], x_d[:, r0 * W : r0 * W + n])
            xb = bfpool.tile([128, CH * W], bf16, tag="xb")
            nc.scalar.copy(xb[:, 0:n], t[:, 0:n])
            chunks[c] = xb

        # per-tap column windows: out[:, so0:so1] += W_kx^T @ in[:, si0:si1]
        CUTS = {0: (0, 511, 1, 512), 1: (0, 512, 0, 512), 2: (1, 512, 0, 511)}

        def row_slice(j, si0, si1):
            t = chunks[j // CH]
            o = (j % CH) * W
            return t[:, o + si0 : o + si1]

        loaded = 0
        for g in range(HS // 8):  # groups of 4 row-pairs
            S = opool.tile([40, 4 * W], f32, tag="S")
            for i in range(4):
                p = 4 * g + i
                cneed = (2 * p + 3) // CH
                while loaded <= min(cneed + 4, NCHUNK - 1):
                    load_chunk(loaded)
                    loaded += 1

                P = ppool.tile([128, W], f32, tag="pp")
                a, b, c, d = 2 * p, 2 * p + 1, 2 * p + 2, 2 * p + 3
                first = True
                # full-width matmuls: rows b ([W1|W0]) and c ([W2|W1])
                for kx in (1, 0, 2):
                    si0, si1, so0, so1 = CUTS[kx]
                    for t_idx, j in ((0, b), (1, c)):
                        blk = 2 * kx + t_idx
                        nc.tensor.matmul(
                            P[:, so0:so1],
                            wf[:, blk * 128 : (blk + 1) * 128],
                            row_slice(j, si0, si1),
                            start=first,
                            stop=False,
                        )
                        first = False
                # half matmuls: row a (W0 -> psum[0:64]), row d (W2 -> psum[64:128])
                for kx in (1, 0, 2):
                    si0, si1, so0, so1 = CUTS[kx]
                    last = kx == 2
                    nc.tensor.matmul(
                        P[0:64, so0:so1],
                        wh[:, (2 * kx) * 64 : (2 * kx + 1) * 64],
                        row_slice(a, si0, si1),
                        start=False,
                        stop=last,
                    )
                    nc.tensor.matmul(
                        P[64:128, so0:so1],
                        wh[:, (2 * kx + 1) * 64 : (2 * kx + 2) * 64],
                        row_slice(d, si0, si1),
                        start=False,
                        stop=last,
                    )

                xr = xrpool.tile([128, W], bf16, tag="xr")
                nc.vector.tensor_scalar_max(xr[:], P[:], 0.0)

                hP = hpool.tile([40, W], f32, tag="hp")
                nc.tensor.matmul(hP[:], whd[:], xr[:], start=True, stop=True)

                nc.vector.tensor_scalar_add(S[:, i * W : (i + 1) * W], hP[:], bt[:])
                nc.scalar.activation(
                    S[32:40, i * W : (i + 1) * W], hP[32:40, :], Sigmoid, bias=bt[32:40, :]
                )

            # 4 batched output DMAs per group (even/odd rows x nonscr/scr),
            # issued on the idle gpsimd engine (SWDGE) to keep the sync queue free for
            # input prefetch.
            y0 = 8 * g
            Sv = S[:].rearrange("q (i w) -> q i w", w=W)  # [40, 4, 512]
            ov = out_d[:, y0 * W : (y0 + 8) * W].rearrange(
                "q (i e w) -> q i e w", i=4, w=W
            )  # [20, 4, 2, 512]
            nc.gpsimd.dma_start(ov[0:16, :, 0, :], Sv[0:16])
            nc.gpsimd.dma_start(ov[0:16, :, 1, :], Sv[16:32])
            nc.gpsimd.dma_start(ov[16:20, :, 0, :], Sv[32:36])
            nc.gpsimd.dma_start(ov[16:20, :, 1, :], Sv[36:40])

    nc.compile()
    return nc


def _get_nc():
    if "nc" not in _NC_CACHE:
        _NC_CACHE["nc"] = _build_nc()
    return _NC_CACHE["nc"]


def _pack_weights(w_shared, w_cls, b_cls, w_box, b_box, w_dir, b_dir, w_scr, b_scr):
    Wt = np.ascontiguousarray(w_shared, np.float32).transpose(1, 0, 2, 3)  # [128,64,3,3]
    wfull = np.zeros((128, 6, 128), np.float32)
    whalf = np.zeros((128, 6, 64), np.float32)
    for kx in range(3):
        wfull[:, 2 * kx + 0, 0:64] = Wt[:, :, 1, kx]
        wfull[:, 2 * kx + 0, 64:128] = Wt[:, :, 0, kx]
        wfull[:, 2 * kx + 1, 0:64] = Wt[:, :, 2, kx]
        wfull[:, 2 * kx + 1, 64:128] = Wt[:, :, 1, kx]
        whalf[:, 2 * kx + 0] = Wt[:, :, 0, kx]
        whalf[:, 2 * kx + 1] = Wt[:, :, 2, kx]
    import ml_dtypes

    wfull = np.ascontiguousarray(wfull.reshape(128, 768)).astype(ml_dtypes.bfloat16)
    whalf = np.ascontiguousarray(whalf.reshape(128, 384)).astype(ml_dtypes.bfloat16)

    Wh = np.concatenate([w_cls, w_box, w_dir, w_scr], 0)[:, :, 0, 0].astype(np.float32)  # [20,64]
    bh = np.concatenate([b_cls, b_box, b_dir, b_scr], 0).astype(np.float32)  # [20]
    # head-output partition layout (sigmoid rows 32-aligned for ACT):
    #   0:16  row-y   cls/box/dir        (k 0:64)
    #   16:32 row-y+1 cls/box/dir        (k 64:128)
    #   32:36 row-y   scr                (k 0:64)
    #   36:40 row-y+1 scr                (k 64:128)
    wheads = np.zeros((128, 40), np.float32)
    wheads[0:64, 0:16] = Wh[0:16].T
    wheads[64:128, 16:32] = Wh[0:16].T
    wheads[0:64, 32:36] = Wh[16:20].T
    wheads[64:128, 36:40] = Wh[16:20].T
    wheads = wheads.astype(ml_dtypes.bfloat16)
    b40 = np.ascontiguousarray(
        np.concatenate([bh[0:16], bh[0:16], bh[16:20], bh[16:20]])[:, None]
    )  # [40,1]
    return wfull, whalf, wheads, b40


def kernel(**inputs):
    from concourse.bass_utils import run_bass_kernel_spmd

    feature = np.ascontiguousarray(inputs["feature"], np.float32)  # [4,128,512,512]
    B, Cin, H, Wd = feature.shape
    assert (B, Cin, H, Wd) == (4, 128, 512, 512)

    wfull, whalf, wheads, b40 = _pack_weights(
        np.asarray(inputs["w_shared"]),
        np.asarray(inputs["w_cls"]), np.asarray(inputs["b_cls"]),
        np.asarray(inputs["w_box"]), np.asarray(inputs["b_box"]),
        np.asarray(inputs["w_dir"]), np.asarray(inputs["b_dir"]),
        np.asarray(inputs["w_scr"]), np.asarray(inputs["b_scr"]),
    )

    in_maps = []
    for core in range(8):
        bi, half = core // 2, core % 2
        r0 = half * HS
        xs = np.zeros((128, HALO, W), np.float32)
        lo, hi = r0 - 1, r0 + HS + 1
        slo, shi = max(lo, 0), min(hi, H)
        xs[:, slo - lo : HALO - (hi - shi), :] = feature[bi, :, slo:shi, :]
        in_maps.append(
            {
                "x": xs.reshape(128, HALO * W),
                "wfull": wfull,
                "whalf": whalf,
                "wheads": wheads,
                "b40": b40,
            }
        )

    nc = _get_nc()
    res = run_bass_kernel_spmd(nc, in_maps, core_ids=list(range(8)))

    out = np.empty((4, 20, 512, 512), np.float32)
    for core in range(8):
        bi, half = core // 2, core % 2
        out[bi, :, half * HS : (half + 1) * HS, :] = res.results[core]["out"].reshape(
            20, HS, W
        )
    return out


def run_traced(**inputs):
    """Like kernel(), but returns (out, BassKernelResults) with a profile trace."""
    from concourse.bass_utils import run_bass_kernel_spmd

    feature = np.ascontiguousarray(inputs["feature"], np.float32)
    wfull, whalf, wheads, b40 = _pack_weights(
        np.asarray(inputs["w_shared"]),
        np.asarray(inputs["w_cls"]), np.asarray(inputs["b_cls"]),
        np.asarray(inputs["w_box"]), np.asarray(inputs["b_box"]),
        np.asarray(inputs["w_dir"]), np.asarray(inputs["b_dir"]),
        np.asarray(inputs["w_scr"]), np.asarray(inputs["b_scr"]),
    )
    in_maps = []
    for core in range(8):
        bi, half = core // 2, core % 2
        r0 = half * HS
        xs = np.zeros((128, HALO, W), np.float32)
        lo, hi = r0 - 1, r0 + HS + 1
        slo, shi = max(lo, 0), min(hi, 512)
        xs[:, slo - lo : HALO - (hi - shi), :] = feature[bi, :, slo:shi, :]
        in_maps.append(
            {"x": xs.reshape(128, HALO * W), "wfull": wfull, "whalf": whalf,
             "wheads": wheads, "b40": b40}
        )
    nc = _get_nc()
    res = run_bass_kernel_spmd(nc, in_maps, core_ids=list(range(8)), trace=True)
    out = np.empty((4, 20, 512, 512), np.float32)
    for core in range(8):
        bi, half = core // 2, core % 2
        out[bi, :, half * HS : (half + 1) * HS, :] = res.results[core]["out"].reshape(
            20, HS, W
        )
    return out, res


# revision 20
# speedup vs baseline: 1.8843x; 1.0333x over previous
"""Trainium2 Bass kernel for DetCenterDense: shared 3x3 conv + ReLU + four 1x1
head convs (cls/box/dir/scr, sigmoid on scr), output concatenated on channels.

Full inputs in / full output out. Sharding: 8 cores = batch(4) x H-halves(2).
Each core computes a [20, 256, 512] output shard from a [128, 258, 512]
haloed input shard.

Per-core compute: the 3x3 conv is 9 shifted 1x1 convs accumulated in PSUM.
Output rows are processed in pairs packed into one PSUM tile [128, 512]
(partitions 0:64 = row y, 64:128 = row y+1), which lets most matmuls run with
M=128 (packed weights [W_a | W_b]) instead of M=64, and gives the head conv a
full K=128 contraction via block-diagonal head weights.
"""

import numpy as np

HS = 256          # output rows per core shard
HALO = HS + 2     # input rows per core shard (1-row halo each side)
W = 512
CH = 4            # input rows per DMA chunk
NCHUNK = (HALO + CH - 1) // CH

_NC_CACHE = {}


def _build_nc():
    from contextlib import ExitStack

    import concourse.mybir as mybir
    import concourse.tile as tile
    from concourse import bacc

    f32 = mybir.dt.float32
    bf16 = mybir.dt.bfloat16
    Sigmoid = mybir.ActivationFunctionType.Sigmoid

    nc = bacc.Bacc("TRN2", target_bir_lowering=False, debug=False, num_devices=8)
    x_d = nc.dram_tensor("x", [128, HALO * W], bf16, kind="ExternalInput").ap()
    wf_d = nc.dram_tensor("wfull", [128, 6 * 128], bf16, kind="ExternalInput").ap()
    wh_d = nc.dram_tensor("whalf", [128, 6 * 64], bf16, kind="ExternalInput").ap()
    whd_d = nc.dram_tensor("wheads", [128, 40], bf16, kind="ExternalInput").ap()
    b_d = nc.dram_tensor("b40", [40, 1], f32, kind="ExternalInput").ap()
    out_d = nc.dram_tensor("out", [20, HS * W], f32, kind="ExternalOutput").ap()

    with ExitStack() as ctx:
        tc = ctx.enter_context(tile.TileContext(nc))
        wpool = ctx.enter_context(tc.tile_pool(name="w", bufs=1))
        bfpool = ctx.enter_context(tc.tile_pool(name="xbf", bufs=12))
        xrpool = ctx.enter_context(tc.tile_pool(name="xr", bufs=3))
        opool = ctx.enter_context(tc.tile_pool(name="ot", bufs=3))
        ppool = ctx.enter_context(tc.tile_pool(name="pp", bufs=3, space="PSUM"))
        hpool = ctx.enter_context(tc.tile_pool(name="hp", bufs=2, space="PSUM"))

        wf = wpool.tile([128, 6 * 128], bf16)
        nc.sync.dma_start(wf[:], wf_d[:])
        wh = wpool.tile([128, 6 * 64], bf16)
        nc.sync.dma_start(wh[:], wh_d[:])
        whd = wpool.tile([128, 40], bf16)
        nc.sync.dma_start(whd[:], whd_d[:])
        bt = wpool.tile([40, 1], f32)
        nc.sync.dma_start(bt[:], b_d[:])

        chunks = [None] * NCHUNK

        def load_chunk(c):
            r0 = c * CH
            rows = min(CH, HALO - r0)
            n = rows * W
            xb = bfpool.tile([128, CH * W], bf16, tag="xb")
            nc.sync.dma_start(xb[:, 0:n], x_d[:, r0 * W : r0 * W + n])
            chunks[c] = xb

        # per-tap column windows: out[:, so0:so1] += W_kx^T @ in[:, si0:si1]
        CUTS = {0: (0, 511, 1, 512), 1: (0, 512, 0, 512), 2: (1, 512, 0, 511)}

        def row_slice(j, si0, si1):
            t = chunks[j // CH]
            o = (j % CH) * W
            return t[:, o + si0 : o + si1]

        loaded = 0
        for g in range(HS // 8):  # groups of 4 row-pairs
            S = opool.tile([40, 4 * W], f32, tag="S")
            for i in range(4):
                p = 4 * g + i
                cneed = (2 * p + 3) // CH
                while loaded <= min(cneed + 6, NCHUNK - 1):
                    load_chunk(loaded)
                    loaded += 1

                P = ppool.tile([128, W], f32, tag="pp")
                a, b, c, d = 2 * p, 2 * p + 1, 2 * p + 2, 2 * p + 3
                first = True
                # full-width matmuls: rows b ([W1|W0]) and c ([W2|W1])
                for kx in (1, 0, 2):
                    si0, si1, so0, so1 = CUTS[kx]
                    for t_idx, j in ((0, b), (1, c)):
                        blk = 2 * kx + t_idx
                        nc.tensor.matmul(
                            P[:, so0:so1],
                            wf[:, blk * 128 : (blk + 1) * 128],
                            row_slice(j, si0, si1),
                            start=first,
                            stop=False,
                        )
                        first = False
                # half matmuls: row a (W0 -> psum[0:64]), row d (W2 -> psum[64:128])
                for kx in (1, 0, 2):
                    si0, si1, so0, so1 = CUTS[kx]
                    last = kx == 2
                    nc.tensor.matmul(
                        P[0:64, so0:so1],
                        wh[:, (2 * kx) * 64 : (2 * kx + 1) * 64],
                        row_slice(a, si0, si1),
                        start=False,
                        stop=last,
                    )
                    nc.tensor.matmul(
                        P[64:128, so0:so1],
                        wh[:, (2 * kx + 1) * 64 : (2 * kx + 2) * 64],
                        row_slice(d, si0, si1),
                        start=False,
                        stop=last,
                    )

                xr = xrpool.tile([128, W], bf16, tag="xr")
                nc.vector.tensor_scalar_max(xr[:], P[:], 0.0)

                hP = hpool.tile([40, W], f32, tag="hp")
                nc.tensor.matmul(hP[:], whd[:], xr[:], start=True, stop=True)

                nc.vector.tensor_scalar_add(S[:, i * W : (i + 1) * W], hP[:], bt[:])
                nc.scalar.activation(
                    S[32:40, i * W : (i + 1) * W], hP[32:40, :], Sigmoid, bias=bt[32:40, :]
                )

            # 4 batched output DMAs per group (even/odd rows x nonscr/scr),
            # issued on the idle gpsimd engine (SWDGE) to keep the sync queue free for
            # input prefetch.
            y0 = 8 * g
            Sv = S[:].rearrange("q (i w) -> q i w", w=W)  # [40, 4, 512]
            ov = out_d[:, y0 * W : (y0 + 8) * W].rearrange(
                "q (i e w) -> q i e w", i=4, w=W
            )  # [20, 4, 2, 512]
            nc.gpsimd.dma_start(ov[0:16, :, 0, :], Sv[0:16])
            nc.gpsimd.dma_start(ov[0:16, :, 1, :], Sv[16:32])
            nc.gpsimd.dma_start(ov[16:20, :, 0, :], Sv[32:36])
            nc.gpsimd.dma_start(ov[16:20, :, 1, :], Sv[36:40])

    nc.compile()
    return nc


def _get_nc():
    if "nc" not in _NC_CACHE:
        _NC_CACHE["nc"] = _build_nc()
    return _NC_CACHE["nc"]


def _pack_weights(w_shared, w_cls, b_cls, w_box, b_box, w_dir, b_dir, w_scr, b_scr):
    Wt = np.ascontiguousarray(w_shared, np.float32).transpose(1, 0, 2, 3)  # [128,64,3,3]
    wfull = np.zeros((128, 6, 128), np.float32)
    whalf = np.zeros((128, 6, 64), np.float32)
    for kx in range(3):
        wfull[:, 2 * kx + 0, 0:64] = Wt[:, :, 1, kx]
        wfull[:, 2 * kx + 0, 64:128] = Wt[:, :, 0, kx]
        wfull[:, 2 * kx + 1, 0:64] = Wt[:, :, 2, kx]
        wfull[:, 2 * kx + 1, 64:128] = Wt[:, :, 1, kx]
        whalf[:, 2 * kx + 0] = Wt[:, :, 0, kx]
        whalf[:, 2 * kx + 1] = Wt[:, :, 2, kx]
    import ml_dtypes

    wfull = np.ascontiguousarray(wfull.reshape(128, 768)).astype(ml_dtypes.bfloat16)
    whalf = np.ascontiguousarray(whalf.reshape(128, 384)).astype(ml_dtypes.bfloat16)

    Wh = np.concatenate([w_cls, w_box, w_dir, w_scr], 0)[:, :, 0, 0].astype(np.float32)  # [20,64]
    bh = np.concatenate([b_cls, b_box, b_dir, b_scr], 0).astype(np.float32)  # [20]
    # head-output partition layout (sigmoid rows 32-aligned for ACT):
    #   0:16  row-y   cls/box/dir        (k 0:64)
    #   16:32 row-y+1 cls/box/dir        (k 64:128)
    #   32:36 row-y   scr                (k 0:64)
    #   36:40 row-y+1 scr                (k 64:128)
    wheads = np.zeros((128, 40), np.float32)
    wheads[0:64, 0:16] = Wh[0:16].T
    wheads[64:128, 16:32] = Wh[0:16].T
    wheads[0:64, 32:36] = Wh[16:20].T
    wheads[64:128, 36:40] = Wh[16:20].T
    wheads = wheads.astype(ml_dtypes.bfloat16)
    b40 = np.ascontiguousarray(
        np.concatenate([bh[0:16], bh[0:16], bh[16:20], bh[16:20]])[:, None]
    )  # [40,1]
    return wfull, whalf, wheads, b40


def kernel(**inputs):
    import ml_dtypes

    from concourse.bass_utils import run_bass_kernel_spmd

    feature = np.ascontiguousarray(inputs["feature"], np.float32)  # [4,128,512,512]
    B, Cin, H, Wd = feature.shape
    assert (B, Cin, H, Wd) == (4, 128, 512, 512)

    wfull, whalf, wheads, b40 = _pack_weights(
        np.asarray(inputs["w_shared"]),
        np.asarray(inputs["w_cls"]), np.asarray(inputs["b_cls"]),
        np.asarray(inputs["w_box"]), np.asarray(inputs["b_box"]),
        np.asarray(inputs["w_dir"]), np.asarray(inputs["b_dir"]),
        np.asarray(inputs["w_scr"]), np.asarray(inputs["b_scr"]),
    )

    in_maps = []
    for core in range(8):
        bi, half = core // 2, core % 2
        r0 = half * HS
        xs = np.zeros((128, HALO, W), ml_dtypes.bfloat16)
        lo, hi = r0 - 1, r0 + HS + 1
        slo, shi = max(lo, 0), min(hi, H)
        xs[:, slo - lo : HALO - (hi - shi), :] = feature[bi, :, slo:shi, :].astype(
            ml_dtypes.bfloat16
        )
        in_maps.append(
            {
                "x": xs.reshape(128, HALO * W),
                "wfull": wfull,
                "whalf": whalf,
                "wheads": wheads,
                "b40": b40,
            }
        )

    nc = _get_nc()
    res = run_bass_kernel_spmd(nc, in_maps, core_ids=list(range(8)))

    out = np.empty((4, 20, 512, 512), np.float32)
    for core in range(8):
        bi, half = core // 2, core % 2
        out[bi, :, half * HS : (half + 1) * HS, :] = res.results[core]["out"].reshape(
            20, HS, W
        )
    return out


def run_traced(**inputs):
    """Like kernel(), but returns (out, BassKernelResults) with a profile trace."""
    import ml_dtypes

    from concourse.bass_utils import run_bass_kernel_spmd

    feature = np.ascontiguousarray(inputs["feature"], np.float32)
    wfull, whalf, wheads, b40 = _pack_weights(
        np.asarray(inputs["w_shared"]),
        np.asarray(inputs["w_cls"]), np.asarray(inputs["b_cls"]),
        np.asarray(inputs["w_box"]), np.asarray(inputs["b_box"]),
        np.asarray(inputs["w_dir"]), np.asarray(inputs["b_dir"]),
        np.asarray(inputs["w_scr"]), np.asarray(inputs["b_scr"]),
    )
    in_maps = []
    for core in range(8):
        bi, half = core // 2, core % 2
        r0 = half * HS
        xs = np.zeros((128, HALO, W), ml_dtypes.bfloat16)
        lo, hi = r0 - 1, r0 + HS + 1
        slo, shi = max(lo, 0), min(hi, 512)
        xs[:, slo - lo : HALO - (hi - shi), :] = feature[bi, :, slo:shi, :].astype(
            ml_dtypes.bfloat16
        )
        in_maps.append(
            {"x": xs.reshape(128, HALO * W), "wfull": wfull, "whalf": whalf,
             "wheads": wheads, "b40": b40}
        )
    nc = _get_nc()
    res = run_bass_kernel_spmd(nc, in_maps, core_ids=list(range(8)), trace=True)
    out = np.empty((4, 20, 512, 512), np.float32)
    for core in range(8):
        bi, half = core // 2, core % 2
        out[bi, :, half * HS : (half + 1) * HS, :] = res.results[core]["out"].reshape(
            20, HS, W
        )
    return out, res


# revision 21
# speedup vs baseline: 1.9653x; 1.0430x over previous
"""Trainium2 Bass kernel for DetCenterDense: shared 3x3 conv + ReLU + four 1x1
head convs (cls/box/dir/scr, sigmoid on scr), output concatenated on channels.

Full inputs in / full output out. Sharding: 8 cores = batch(4) x H-halves(2).
Each core computes a [20, 256, 512] output shard from a [128, 258, 512]
haloed input shard.

Per-core compute: the 3x3 conv is 9 shifted 1x1 convs accumulated in PSUM.
Output rows are processed in pairs packed into one PSUM tile [128, 512]
(partitions 0:64 = row y, 64:128 = row y+1), which lets most matmuls run with
M=128 (packed weights [W_a | W_b]) instead of M=64, and gives the head conv a
full K=128 contraction via block-diagonal head weights.
"""

import numpy as np

HS = 256          # output rows per core shard
HALO = HS + 2     # input rows per core shard (1-row halo each side)
W = 512
CH = 4            # input rows per DMA chunk
NCHUNK = (HALO + CH - 1) // CH

_NC_CACHE = {}


def _build_nc():
    from contextlib import ExitStack

    import concourse.mybir as mybir
    import concourse.tile as tile
    from concourse import bacc

    f32 = mybir.dt.float32
    bf16 = mybir.dt.bfloat16
    Sigmoid = mybir.ActivationFunctionType.Sigmoid

    nc = bacc.Bacc("TRN2", target_bir_lowering=False, debug=False, num_devices=8)
    x_d = nc.dram_tensor("x", [128, HALO * W], bf16, kind="ExternalInput").ap()
    wf_d = nc.dram_tensor("wfull", [128, 6 * 128], bf16, kind="ExternalInput").ap()
    wh_d = nc.dram_tensor("whalf", [128, 6 * 64], bf16, kind="ExternalInput").ap()
    whd_d = nc.dram_tensor("wheads", [128, 40], bf16, kind="ExternalInput").ap()
    b_d = nc.dram_tensor("b40", [40, 1], f32, kind="ExternalInput").ap()
    out_d = nc.dram_tensor("out", [20, HS * W], f32, kind="ExternalOutput").ap()

    with ExitStack() as ctx:
        tc = ctx.enter_context(tile.TileContext(nc))
        wpool = ctx.enter_context(tc.tile_pool(name="w", bufs=1))
        bfpool = ctx.enter_context(tc.tile_pool(name="xbf", bufs=12))
        xrpool = ctx.enter_context(tc.tile_pool(name="xr", bufs=3))
        opool = ctx.enter_context(tc.tile_pool(name="ot", bufs=3))
        ppool = ctx.enter_context(tc.tile_pool(name="pp", bufs=4, space="PSUM"))
        hpool = ctx.enter_context(tc.tile_pool(name="hp", bufs=4, space="PSUM"))

        wf = wpool.tile([128, 6 * 128], bf16)
        nc.sync.dma_start(wf[:], wf_d[:])
        wh = wpool.tile([128, 6 * 64], bf16)
        nc.sync.dma_start(wh[:], wh_d[:])
        whd = wpool.tile([128, 40], bf16)
        nc.sync.dma_start(whd[:], whd_d[:])
        bt = wpool.tile([40, 1], f32)
        nc.sync.dma_start(bt[:], b_d[:])

        chunks = [None] * NCHUNK

        def load_chunk(c):
            r0 = c * CH
            rows = min(CH, HALO - r0)
            n = rows * W
            xb = bfpool.tile([128, CH * W], bf16, tag="xb")
            nc.sync.dma_start(xb[:, 0:n], x_d[:, r0 * W : r0 * W + n])
            chunks[c] = xb

        # per-tap column windows: out[:, so0:so1] += W_kx^T @ in[:, si0:si1]
        CUTS = {0: (0, 511, 1, 512), 1: (0, 512, 0, 512), 2: (1, 512, 0, 511)}

        def row_slice(j, si0, si1):
            t = chunks[j // CH]
            o = (j % CH) * W
            return t[:, o + si0 : o + si1]

        loaded = 0
        for g in range(HS // 8):  # groups of 4 row-pairs
            S = opool.tile([40, 4 * W], f32, tag="S")
            for i in range(4):
                p = 4 * g + i
                cneed = (2 * p + 3) // CH
                while loaded <= min(cneed + 6, NCHUNK - 1):
                    load_chunk(loaded)
                    loaded += 1

                P = ppool.tile([128, W], f32, tag="pp")
                a, b, c, d = 2 * p, 2 * p + 1, 2 * p + 2, 2 * p + 3
                first = True
                # full-width matmuls: rows b ([W1|W0]) and c ([W2|W1])
                for kx in (1, 0, 2):
                    si0, si1, so0, so1 = CUTS[kx]
                    for t_idx, j in ((0, b), (1, c)):
                        blk = 2 * kx + t_idx
                        nc.tensor.matmul(
                            P[:, so0:so1],
                            wf[:, blk * 128 : (blk + 1) * 128],
                            row_slice(j, si0, si1),
                            start=first,
                            stop=False,
                        )
                        first = False
                # half matmuls: row a (W0 -> psum[0:64]), row d (W2 -> psum[64:128])
                for kx in (1, 0, 2):
                    si0, si1, so0, so1 = CUTS[kx]
                    last = kx == 2
                    nc.tensor.matmul(
                        P[0:64, so0:so1],
                        wh[:, (2 * kx) * 64 : (2 * kx + 1) * 64],
                        row_slice(a, si0, si1),
                        start=False,
                        stop=last,
                    )
                    nc.tensor.matmul(
                        P[64:128, so0:so1],
                        wh[:, (2 * kx + 1) * 64 : (2 * kx + 2) * 64],
                        row_slice(d, si0, si1),
                        start=False,
                        stop=last,
                    )

                xr = xrpool.tile([128, W], bf16, tag="xr")
                nc.vector.tensor_scalar_max(xr[:], P[:], 0.0)

                hP = hpool.tile([40, W], f32, tag="hp")
                nc.tensor.matmul(hP[:], whd[:], xr[:], start=True, stop=True)

                nc.vector.tensor_scalar_add(S[:, i * W : (i + 1) * W], hP[:], bt[:])
                nc.scalar.activation(
                    S[32:40, i * W : (i + 1) * W], hP[32:40, :], Sigmoid, bias=bt[32:40, :]
                )

            # 4 batched output DMAs per group (even/odd rows x nonscr/scr),
            # issued on the idle gpsimd engine (SWDGE) to keep the sync queue free for
            # input prefetch.
            y0 = 8 * g
            Sv = S[:].rearrange("q (i w) -> q i w", w=W)  # [40, 4, 512]
            ov = out_d[:, y0 * W : (y0 + 8) * W].rearrange(
                "q (i e w) -> q i e w", i=4, w=W
            )  # [20, 4, 2, 512]
            nc.gpsimd.dma_start(ov[0:16, :, 0, :], Sv[0:16])
            nc.gpsimd.dma_start(ov[0:16, :, 1, :], Sv[16:32])
            nc.gpsimd.dma_start(ov[16:20, :, 0, :], Sv[32:36])
            nc.gpsimd.dma_start(ov[16:20, :, 1, :], Sv[36:40])

    nc.compile()
    return nc


def _get_nc():
    if "nc" not in _NC_CACHE:
        _NC_CACHE["nc"] = _build_nc()
    return _NC_CACHE["nc"]


def _pack_weights(w_shared, w_cls, b_cls, w_box, b_box, w_dir, b_dir, w_scr, b_scr):
    Wt = np.ascontiguousarray(w_shared, np.float32).transpose(1, 0, 2, 3)  # [128,64,3,3]
    wfull = np.zeros((128, 6, 128), np.float32)
    whalf = np.zeros((128, 6, 64), np.float32)
    for kx in range(3):
        wfull[:, 2 * kx + 0, 0:64] = Wt[:, :, 1, kx]
        wfull[:, 2 * kx + 0, 64:128] = Wt[:, :, 0, kx]
        wfull[:, 2 * kx + 1, 0:64] = Wt[:, :, 2, kx]
        wfull[:, 2 * kx + 1, 64:128] = Wt[:, :, 1, kx]
        whalf[:, 2 * kx + 0] = Wt[:, :, 0, kx]
        whalf[:, 2 * kx + 1] = Wt[:, :, 2, kx]
    import ml_dtypes

    wfull = np.ascontiguousarray(wfull.reshape(128, 768)).astype(ml_dtypes.bfloat16)
    whalf = np.ascontiguousarray(whalf.reshape(128, 384)).astype(ml_dtypes.bfloat16)

    Wh = np.concatenate([w_cls, w_box, w_dir, w_scr], 0)[:, :, 0, 0].astype(np.float32)  # [20,64]
    bh = np.concatenate([b_cls, b_box, b_dir, b_scr], 0).astype(np.float32)  # [20]
    # head-output partition layout (sigmoid rows 32-aligned for ACT):
    #   0:16  row-y   cls/box/dir        (k 0:64)
    #   16:32 row-y+1 cls/box/dir        (k 64:128)
    #   32:36 row-y   scr                (k 0:64)
    #   36:40 row-y+1 scr                (k 64:128)
    wheads = np.zeros((128, 40), np.float32)
    wheads[0:64, 0:16] = Wh[0:16].T
    wheads[64:128, 16:32] = Wh[0:16].T
    wheads[0:64, 32:36] = Wh[16:20].T
    wheads[64:128, 36:40] = Wh[16:20].T
    wheads = wheads.astype(ml_dtypes.bfloat16)
    b40 = np.ascontiguousarray(
        np.concatenate([bh[0:16], bh[0:16], bh[16:20], bh[16:20]])[:, None]
    )  # [40,1]
    return wfull, whalf, wheads, b40


def kernel(**inputs):
    import ml_dtypes

    from concourse.bass_utils import run_bass_kernel_spmd

    feature = np.ascontiguousarray(inputs["feature"], np.float32)  # [4,128,512,512]
    B, Cin, H, Wd = feature.shape
    assert (B, Cin, H, Wd) == (4, 128, 512, 512)

    wfull, whalf, wheads, b40 = _pack_weights(
        np.asarray(inputs["w_shared"]),
        np.asarray(inputs["w_cls"]), np.asarray(inputs["b_cls"]),
        np.asarray(inputs["w_box"]), np.asarray(inputs["b_box"]),
        np.asarray(inputs["w_dir"]), np.asarray(inputs["b_dir"]),
        np.asarray(inputs["w_scr"]), np.asarray(inputs["b_scr"]),
    )

    in_maps = []
    for core in range(8):
        bi, half = core // 2, core % 2
        r0 = half * HS
        xs = np.zeros((128, HALO, W), ml_dtypes.bfloat16)
        lo, hi = r0 - 1, r0 + HS + 1
        slo, shi = max(lo, 0), min(hi, H)
        xs[:, slo - lo : HALO - (hi - shi), :] = feature[bi, :, slo:shi, :].astype(
            ml_dtypes.bfloat16
        )
        in_maps.append(
            {
                "x": xs.reshape(128, HALO * W),
                "wfull": wfull,
                "whalf": whalf,
                "wheads": wheads,
                "b40": b40,
            }
        )

    nc = _get_nc()
    res = run_bass_kernel_spmd(nc, in_maps, core_ids=list(range(8)))

    out = np.empty((4, 20, 512, 512), np.float32)
    for core in range(8):
        bi, half = core // 2, core % 2
        out[bi, :, half * HS : (half + 1) * HS, :] = res.results[core]["out"].reshape(
            20, HS, W
        )
    return out


def run_traced(**inputs):
    """Like kernel(), but returns (out, BassKernelResults) with a profile trace."""
    import ml_dtypes

    from concourse.bass_utils import run_bass_kernel_spmd

    feature = np.ascontiguousarray(inputs["feature"], np.float32)
    wfull, whalf, wheads, b40 = _pack_weights(
        np.asarray(inputs["w_shared"]),
        np.asarray(inputs["w_cls"]), np.asarray(inputs["b_cls"]),
        np.asarray(inputs["w_box"]), np.asarray(inputs["b_box"]),
        np.asarray(inputs["w_dir"]), np.asarray(inputs["b_dir"]),
        np.asarray(inputs["w_scr"]), np.asarray(inputs["b_scr"]),
    )
    in_maps = []
    for core in range(8):
        bi, half = core // 2, core % 2
        r0 = half * HS
        xs = np.zeros((128, HALO, W), ml_dtypes.bfloat16)
        lo, hi = r0 - 1, r0 + HS + 1
        slo, shi = max(lo, 0), min(hi, 512)
        xs[:, slo - lo : HALO - (hi - shi), :] = feature[bi, :, slo:shi, :].astype(
            ml_dtypes.bfloat16
        )
        in_maps.append(
            {"x": xs.reshape(128, HALO * W), "wfull": wfull, "whalf": whalf,
             "wheads": wheads, "b40": b40}
        )
    nc = _get_nc()
    res = run_bass_kernel_spmd(nc, in_maps, core_ids=list(range(8)), trace=True)
    out = np.empty((4, 20, 512, 512), np.float32)
    for core in range(8):
        bi, half = core // 2, core % 2
        out[bi, :, half * HS : (half + 1) * HS, :] = res.results[core]["out"].reshape(
            20, HS, W
        )
    return out, res
